# revision 9
# baseline (speedup 1.0000x reference)
"""Distributed transformer-block kernel for one TRN2 chip (8 NeuronCores).

Reference computation (S=4096, N=1024):
    xn = LayerNorm(x) * g + b
    q,k,v = xn@Wq+bq, xn@Wk+bk, xn@Wv+bv
    w = softmax((k @ q.T) / sqrt(N), axis=-1)
    h = w @ v
    out = leaky_relu(h@W1+b1, 0.1) @ W2 + b2 + xn

Sharding: sequence-parallel. Each core owns 512 rows of x, computes its
q/k/v shard, all-gathers q and v (bf16), then computes its 512-row slice
of attention + FFN fully locally.

Layout strategy (picked so no big transposes are needed):
  - xn kept natural [512,1024] (residual) and transposed once to
    xnT [1024,512] bf16 (feeds every projection).
  - q, k produced transposed ([1024,512]); v produced natural.
  - logits computed TRANSPOSED: wT[j,i] = q_full @ k_loc.T, so the
    attention matmul hT[c,i] = sum_j v[j,c] * exp(wT)[j,i] uses v natural
    as lhsT and wT as rhs, and hT directly feeds FFN1 as rhs.
  - softmax denominator: exp accumulated with DVE, reduced over
    partitions with a ones-vector matmul. 1/sum applied at the FFN2
    epilogue via leaky_relu's positive homogeneity (bias terms are
    carried through scaled by sum via rank-1 matmul augmentation).

SBUF pressure is managed by entering/exiting tile pools at phase
boundaries (non-nested lifetimes, hence manual __enter__/__exit__).
"""

import sys

sys.path.insert(0, "/opt/trn_rl_repo")

import numpy as np
import ml_dtypes

import concourse.bass as bass
from concourse import bacc, tile, mybir
from concourse.bass import ts
from concourse.bass_utils import run_bass_kernel_spmd
from concourse.masks import make_identity

F32 = mybir.dt.float32
BF16 = mybir.dt.bfloat16
AF = mybir.ActivationFunctionType

P = 128
R = 8            # cores
S = 4096         # sequence
N = 1024         # hidden
FF = 4096        # ffn hidden
SL = S // R      # local rows (512)
NK = N // P      # 8 hidden chunks
NI = SL // P     # 4 local row chunks
NJ = S // P      # 32 global row chunks
NF = FF // P     # 32 ffn chunks
SCALE = 1.0 / np.sqrt(N).astype(np.float32)  # 0.03125
EPS = 1e-5

_cached = None


def _build():
    nc = bacc.Bacc("TRN2", target_bir_lowering=False, debug=False, num_devices=R)

    x_e = nc.declare_dram_parameter("x", [SL, N], F32, isOutput=False)
    g_e = nc.declare_dram_parameter("norm_g", [N], F32, isOutput=False)
    bn_e = nc.declare_dram_parameter("norm_b", [N], F32, isOutput=False)
    wq_e = nc.declare_dram_parameter("wq", [N, N], BF16, isOutput=False)
    bq_e = nc.declare_dram_parameter("bq", [N], F32, isOutput=False)
    wk_e = nc.declare_dram_parameter("wk", [N, N], BF16, isOutput=False)
    bk_e = nc.declare_dram_parameter("bk", [N], F32, isOutput=False)
    wv_e = nc.declare_dram_parameter("wv", [N, N], BF16, isOutput=False)
    bv_e = nc.declare_dram_parameter("bv", [N], BF16, isOutput=False)
    w1_e = nc.declare_dram_parameter("w1", [N, FF], BF16, isOutput=False)
    b1_e = nc.declare_dram_parameter("b1", [FF], BF16, isOutput=False)
    w2_e = nc.declare_dram_parameter("w2", [FF, N], BF16, isOutput=False)
    b2_e = nc.declare_dram_parameter("b2", [N], BF16, isOutput=False)
    out_e = nc.declare_dram_parameter("out", [SL, N], F32, isOutput=True)

    # collective bounce buffers
    warm_in = nc.dram_tensor("warm_in", [1, 128], F32)
    warm_out = nc.dram_tensor("warm_out", [R, 128], F32, addr_space="Shared")
    agq_in = nc.dram_tensor("agq_in", [NK, P, SL], BF16)
    agq_out = nc.dram_tensor("agq_out", [R * NK, P, SL], BF16, addr_space="Shared")
    agv_in = nc.dram_tensor("agv_in", [NI, P, N], BF16)
    agv_out = nc.dram_tensor("agv_out", [R * NI, P, N], BF16, addr_space="Shared")

    rg = [list(range(R))]

    def enter(cm):
        return cm, cm.__enter__()

    def leave(cm):
        cm.__exit__(None, None, None)

    with tile.TileContext(nc) as tc:
        base_cm, base = enter(tc.tile_pool(name="base", bufs=1))

        # ---- whole-kernel constants / carriers ----
        ident = base.tile([P, P], BF16)
        make_identity(nc, ident)
        ones_row_b = base.tile([1, P], BF16)
        nc.gpsimd.memset(ones_row_b[:], 1.0)
        ones_col_f = base.tile([P, 1], F32)
        nc.gpsimd.memset(ones_col_f[:], 1.0)
        one_f = base.tile([1, 1], F32)
        nc.gpsimd.memset(one_f[:], 1.0)
        zero_col = base.tile([P, 1], F32)
        nc.gpsimd.memset(zero_col[:], 0.0)
        eps_col = base.tile([P, 1], F32)
        nc.gpsimd.memset(eps_col[:], EPS)

        xn_sb = base.tile([P, NI * N], BF16)    # normed x, natural layout (residual)
        sum_row_f = base.tile([1, SL], F32)
        sum_row_b = base.tile([1, SL], BF16)
        recip_col = base.tile([P, NI], F32)

        # =========== Phase 0: layernorm + transpose ===========
        xnT_cm, xnTp = enter(tc.tile_pool(name="xnTp", bufs=1, side="left"))
        xnT_sb = xnTp.tile([P, NK * SL], BF16)

        ph0_cm, ph0 = enter(tc.tile_pool(name="ph0", bufs=1, side="left"))
        ones_row_f = ph0.tile([1, P], F32)
        nc.gpsimd.memset(ones_row_f[:], 1.0)
        g_row = ph0.tile([1, N], F32)
        nc.sync.dma_start(g_row[:1, :], g_e[:].rearrange("(a n) -> a n", a=1))
        b_row = ph0.tile([1, N], F32)
        nc.sync.dma_start(b_row[:1, :], bn_e[:].rearrange("(a n) -> a n", a=1))
        g_bcast = ph0.tile([P, N], F32)
        b_bcast = ph0.tile([P, N], F32)
        with tc.tile_pool(name="bpsum", bufs=2, space="PSUM") as bpsum:
            for vec_row, bcast in ((g_row, g_bcast), (b_row, b_bcast)):
                for blk in range(2):
                    pb = bpsum.tile([P, 512], F32, tag="pb")
                    nc.tensor.matmul(pb[:], ones_row_f[:], vec_row[:1, ts(blk, 512)])
                    nc.vector.tensor_copy(bcast[:, ts(blk, 512)], pb[:])

        with (
            tc.tile_pool(name="xs", bufs=4) as xs,
            tc.tile_pool(name="ln", bufs=4) as ln,
            tc.tile_pool(name="tpsum", bufs=4, space="PSUM") as tpsum,
        ):
            for i in range(NI):
                xt = xs.tile([P, N], F32, tag="xt")
                nc.sync.dma_start(xt[:], x_e[ts(i, P), :])
                sum_t = ln.tile([P, 1], F32, tag="sum")
                nc.vector.reduce_sum(sum_t[:], xt[:], axis=mybir.AxisListType.X)
                sq_scr = xs.tile([P, N], BF16, tag="sq")
                sumsq_t = ln.tile([P, 1], F32, tag="sumsq")
                nc.scalar.activation(sq_scr[:], xt[:], AF.Square, bias=zero_col[:], accum_out=sumsq_t[:])
                mu_t = ln.tile([P, 1], F32, tag="mu")
                nc.vector.tensor_scalar_mul(mu_t[:], sum_t[:], 1.0 / N)
                var_t = ln.tile([P, 1], F32, tag="var")
                nc.vector.tensor_scalar_mul(var_t[:], sumsq_t[:], 1.0 / N)
                musq_t = ln.tile([P, 1], F32, tag="musq")
                nc.vector.tensor_mul(musq_t[:], mu_t[:], mu_t[:])
                nc.vector.tensor_sub(var_t[:], var_t[:], musq_t[:])
                std_t = ln.tile([P, 1], F32, tag="std")
                nc.scalar.activation(std_t[:], var_t[:], AF.Sqrt, bias=eps_col[:])
                rstd_t = ln.tile([P, 1], F32, tag="rstd")
                nc.vector.reciprocal(rstd_t[:], std_t[:])
                nmr_t = ln.tile([P, 1], F32, tag="nmr")
                nc.vector.tensor_mul(nmr_t[:], mu_t[:], rstd_t[:])
                nc.vector.tensor_scalar_mul(nmr_t[:], nmr_t[:], -1.0)
                # f32 staging for the affine, cast to bf16 on the last op
                xnf = xs.tile([P, N], F32, tag="xnf")
                nc.scalar.activation(xnf[:], xt[:], AF.Identity, scale=rstd_t[:], bias=nmr_t[:])
                nc.vector.tensor_mul(xnf[:], xnf[:], g_bcast[:])
                xn_i = xn_sb[:, ts(i, N)]
                nc.vector.tensor_add(xn_i, xnf[:], b_bcast[:])
                for k in range(NK):
                    pt = tpsum.tile([P, P], BF16, tag="pt")
                    nc.tensor.transpose(pt[:], xn_sb[:, i * N + k * P : i * N + (k + 1) * P], ident[:])
                    nc.scalar.activation(
                        xnT_sb[:, k * SL + i * P : k * SL + (i + 1) * P], pt[:], AF.Copy
                    )
        leave(ph0_cm)

        # =========== Phase 1: q/k/v projections + all-gathers ===========
        kT_cm, kTp = enter(tc.tile_pool(name="kTp", bufs=1, side="right"))
        kT_sb = kTp.tile([P, NK * SL], BF16)

        qkv_cm, qkv = enter(tc.tile_pool(name="qkv", bufs=1, side="right"))
        bq_col = qkv.tile([P, NK], F32)
        nc.sync.dma_start(bq_col[:], bq_e[:].rearrange("(m p) -> p m", p=P))
        bk_col = qkv.tile([P, NK], F32)
        nc.sync.dma_start(bk_col[:], bk_e[:].rearrange("(m p) -> p m", p=P))
        bv_row = qkv.tile([1, N], BF16)
        nc.sync.dma_start(bv_row[:1, :], bv_e[:].rearrange("(a n) -> a n", a=1))
        wq_sb = [qkv.tile([P, N], BF16, tag=f"wq{k}", name=f"wq{k}") for k in range(NK)]
        wk_sb = [qkv.tile([P, N], BF16, tag=f"wk{k}", name=f"wk{k}") for k in range(NK)]
        wv_sb = [qkv.tile([P, N], BF16, tag=f"wv{k}", name=f"wv{k}") for k in range(NK)]
        qT_sb = qkv.tile([P, NK * SL], BF16)
        v_sb = qkv.tile([P, NI * N], BF16)
        for k in range(NK):
            nc.scalar.dma_start(wq_sb[k][:], wq_e[ts(k, P), :])
        for k in range(NK):
            nc.scalar.dma_start(wk_sb[k][:], wk_e[ts(k, P), :])
        for k in range(NK):
            nc.scalar.dma_start(wv_sb[k][:], wv_e[ts(k, P), :])

        with tc.tile_pool(name="qpsum", bufs=3, space="PSUM") as qpsum:
            # q (transposed layout), then its all-gather right away
            for m in range(NK):
                pq = qpsum.tile([P, SL], F32, tag="pq")
                for k in range(NK):
                    nc.tensor.matmul(
                        pq[:],
                        wq_sb[k][:, ts(m, P)],
                        xnT_sb[:, ts(k, SL)],
                        start=(k == 0),
                        stop=(k == NK - 1),
                    )
                nc.scalar.activation(
                    qT_sb[:, ts(m, SL)], pq[:], AF.Identity, bias=bq_col[:, m : m + 1]
                )
                nc.gpsimd.dma_start(agq_in[m], qT_sb[:, ts(m, SL)])
            nc.gpsimd.collective_compute(
                "AllGather", mybir.AluOpType.bypass, replica_groups=rg,
                ins=[agq_in[:]], outs=[agq_out[:]],
            )

            # k (transposed layout, stays local)
            for m in range(NK):
                pk = qpsum.tile([P, SL], F32, tag="pq")
                for k in range(NK):
                    nc.tensor.matmul(
                        pk[:],
                        wk_sb[k][:, ts(m, P)],
                        xnT_sb[:, ts(k, SL)],
                        start=(k == 0),
                        stop=(k == NK - 1),
                    )
                nc.scalar.activation(
                    kT_sb[:, ts(m, SL)], pk[:], AF.Identity, bias=bk_col[:, m : m + 1]
                )

            # v (natural layout) + its all-gather
            for i in range(NI):
                for cb in range(2):
                    pv = qpsum.tile([P, 512], F32, tag="pq")
                    nc.tensor.matmul(
                        pv[:], ones_row_b[:], bv_row[:1, ts(cb, 512)],
                        start=True, stop=False,
                    )
                    for k in range(NK):
                        nc.tensor.matmul(
                            pv[:],
                            xnT_sb[:, k * SL + i * P : k * SL + (i + 1) * P],
                            wv_sb[k][:, ts(cb, 512)],
                            start=False,
                            stop=(k == NK - 1),
                        )
                    nc.scalar.activation(
                        v_sb[:, i * N + cb * 512 : i * N + (cb + 1) * 512], pv[:], AF.Copy
                    )
                nc.gpsimd.dma_start(agv_in[i], v_sb[:, ts(i, N)])
            nc.gpsimd.collective_compute(
                "AllGather", mybir.AluOpType.bypass, replica_groups=rg,
                ins=[agv_in[:]], outs=[agv_out[:]],
            )
        leave(qkv_cm)
        leave(xnT_cm)

        # W1 resident; emitted here so it prefetches during attention
        w1_cm, w1p = enter(tc.tile_pool(name="w1p", bufs=1, side="left"))
        w1_sb = [w1p.tile([P, FF], BF16, tag=f"w1{c}", name=f"w1{c}") for c in range(NK)]
        for c in range(NK):
            nc.gpsimd.dma_start(w1_sb[c][:], w1_e[ts(c, P), :])
        b1_row = w1p.tile([1, FF], BF16)
        nc.gpsimd.dma_start(b1_row[:1, :], b1_e[:].rearrange("(a n) -> a n", a=1))

        # =========== Phase 2: logits (transposed) + exp + running sum ===========
        wT_cm, wTp = enter(tc.tile_pool(name="wTp", bufs=1, side="left"))
        wT_sb = wTp.tile([P, NJ * SL], BF16)
        acc = wTp.tile([P, SL], F32)
        nc.vector.memset(acc[:], 0.0)
        with (
            tc.tile_pool(name="qf", bufs=2) as qfp,
            tc.tile_pool(name="wpsum", bufs=3, space="PSUM") as wpsum,
        ):
            for rank in range(R):
                qf = qfp.tile([P, NK * SL], BF16, tag="qf")
                for n in range(NK):
                    nc.sync.dma_start(qf[:, ts(n, SL)], agq_out[rank * NK + n])
                for sub in range(NI):
                    jc = rank * NI + sub
                    pw = wpsum.tile([P, SL], F32, tag="pw")
                    for n in range(NK):
                        nc.tensor.matmul(
                            pw[:],
                            qf[:, n * SL + sub * P : n * SL + (sub + 1) * P],
                            kT_sb[:, ts(n, SL)],
                            start=(n == 0),
                            stop=(n == NK - 1),
                        )
                    nc.scalar.activation(
                        wT_sb[:, ts(jc, SL)], pw[:], AF.Exp, scale=float(SCALE), bias=zero_col[:]
                    )
                    nc.vector.tensor_add(acc[:], acc[:], wT_sb[:, ts(jc, SL)])
        leave(kT_cm)

        # =========== Phase 3: hT accumulation over all j ===========
        mid_cm, midp = enter(tc.tile_pool(name="midp", bufs=1, side="right"))
        hT_sb = midp.tile([P, NK * SL], BF16)
        ff1T_sb = midp.tile([P, NF * SL], BF16)
        with (
            tc.tile_pool(name="vstream", bufs=6) as vsp,
            tc.tile_pool(name="hpsum", bufs=1, space="PSUM") as hpsum,
        ):
            ph = [hpsum.tile([P, SL], F32, tag=f"ph{c}", name=f"ph{c}") for c in range(NK)]
            for j in range(NJ):
                vt = vsp.tile([P, N], BF16, tag="vt")
                nc.sync.dma_start(vt[:], agv_out[j])
                for c in range(NK):
                    nc.tensor.matmul(
                        ph[c][:],
                        vt[:, ts(c, P)],
                        wT_sb[:, ts(j, SL)],
                        start=(j == 0),
                        stop=(j == NJ - 1),
                    )
            for c in range(NK):
                if c % 2 == 0:
                    nc.scalar.activation(hT_sb[:, ts(c, SL)], ph[c][:], AF.Copy)
                else:
                    nc.vector.tensor_copy(hT_sb[:, ts(c, SL)], ph[c][:])
        # sumexp finalize: PE cost is tiny and overlaps the hT evacuations
        with tc.tile_pool(name="spsum", bufs=2, space="PSUM") as spsum:
            ps = spsum.tile([1, SL], F32, tag="ps")
            nc.tensor.matmul(ps[:], ones_col_f[:], acc[:])
            nc.vector.tensor_copy(sum_row_f[:1, :], ps[:1, :])
            nc.scalar.activation(sum_row_b[:1, :], ps[:1, :], AF.Copy)
            for ic in range(NI):
                pr = spsum.tile([P, 1], F32, tag="pr")
                nc.tensor.matmul(pr[:], sum_row_f[:1, ts(ic, P)], one_f[:1, :])
                nc.vector.reciprocal(recip_col[:, ic : ic + 1], pr[:])
        leave(wT_cm)

        # =========== Phase 4: FFN1 (transposed out, leaky via homogeneity) ===========
        with tc.tile_pool(name="fpsum", bufs=3, space="PSUM") as fpsum:
            for f in range(NF):
                pf = fpsum.tile([P, SL], F32, tag="pf")
                nc.tensor.matmul(
                    pf[:], b1_row[:1, ts(f, P)], sum_row_b[:1, :],
                    start=True, stop=False,
                )
                for c in range(NK):
                    nc.tensor.matmul(
                        pf[:],
                        w1_sb[c][:, ts(f, P)],
                        hT_sb[:, ts(c, SL)],
                        start=False,
                        stop=(c == NK - 1),
                    )
                nc.scalar.activation(ff1T_sb[:, ts(f, SL)], pf[:], AF.Lrelu, alpha=0.1, bias=zero_col[:])
        leave(w1_cm)

        # =========== Phase 5: FFN2 + epilogue (scale, bias, residual) ===========
        with (
            tc.tile_pool(name="ph5", bufs=1) as ph5,
            tc.tile_pool(name="w2s", bufs=8) as w2s,
            tc.tile_pool(name="outp", bufs=3) as outp,
            tc.tile_pool(name="opsum", bufs=1, space="PSUM") as opsum,
        ):
            b2_row = ph5.tile([1, N], BF16)
            nc.sync.dma_start(b2_row[:1, :], b2_e[:].rearrange("(a n) -> a n", a=1))
            po = [
                opsum.tile([P, 512], F32, tag=f"po{i}", name=f"po{i}")
                for i in range(NI * 2)
            ]
            for ic in range(NI):
                for mb in range(2):
                    nc.tensor.matmul(
                        po[ic * 2 + mb][:],
                        sum_row_b[:1, ts(ic, P)],
                        b2_row[:1, ts(mb, 512)],
                        start=True, stop=False,
                    )
            for f in range(NF):
                w2t = w2s.tile([P, N], BF16, tag="w2t")
                nc.sync.dma_start(w2t[:], w2_e[ts(f, P), :])
                for ic in range(NI):
                    for mb in range(2):
                        nc.tensor.matmul(
                            po[ic * 2 + mb][:],
                            ff1T_sb[:, f * SL + ic * P : f * SL + (ic + 1) * P],
                            w2t[:, ts(mb, 512)],
                            start=False,
                            stop=(f == NF - 1),
                        )
            for ic in range(NI):
                for mb in range(2):
                    ot = outp.tile([P, 512], F32, tag="ot")
                    nc.vector.scalar_tensor_tensor(
                        ot[:],
                        po[ic * 2 + mb][:],
                        recip_col[:, ic : ic + 1],
                        xn_sb[:, ic * N + mb * 512 : ic * N + (mb + 1) * 512],
                        op0=mybir.AluOpType.mult,
                        op1=mybir.AluOpType.add,
                    )
                    nc.sync.dma_start(out_e[ts(ic, P), ts(mb, 512)], ot[:])
        leave(mid_cm)
        leave(base_cm)

    nc.compile()
    return nc


def _get_nc():
    global _cached
    if _cached is None:
        _cached = _build()
    return _cached


def kernel(**inputs):
    nc = _get_nc()
    bf = lambda a: np.asarray(a, dtype=np.float32).astype(ml_dtypes.bfloat16)
    f = lambda a: np.ascontiguousarray(np.asarray(a, dtype=np.float32))
    x = f(inputs["x"])
    common = {
        "norm_g": f(inputs["norm_g"]),
        "norm_b": f(inputs["norm_b"]),
        "wq": bf(inputs["Wq"]),
        "bq": f(inputs["bq"]),
        "wk": bf(inputs["Wk"]),
        "bk": f(inputs["bk"]),
        "wv": bf(inputs["Wv"]),
        "bv": bf(inputs["bv"]),
        "w1": bf(inputs["W1"]),
        "b1": bf(inputs["b1"]),
        "w2": bf(inputs["W2"]),
        "b2": bf(inputs["b2"]),
    }
    in_maps = [dict(common, x=np.ascontiguousarray(x[r * SL : (r + 1) * SL])) for r in range(R)]
    res = run_bass_kernel_spmd(nc, in_maps, list(range(R)))
    return np.concatenate([res.results[r]["out"] for r in range(R)], axis=0)


if __name__ == "__main__":
    rng = np.random.default_rng(0)
    demo = {
        "x": rng.standard_normal((S, N), dtype=np.float32),
        "norm_g": np.ones(N, np.float32),
        "norm_b": np.zeros(N, np.float32),
        "Wq": rng.standard_normal((N, N), dtype=np.float32) * SCALE,
        "bq": np.zeros(N, np.float32),
        "Wk": rng.standard_normal((N, N), dtype=np.float32) * SCALE,
        "bk": np.zeros(N, np.float32),
        "Wv": rng.standard_normal((N, N), dtype=np.float32) * SCALE,
        "bv": np.zeros(N, np.float32),
        "W1": rng.standard_normal((N, FF), dtype=np.float32) * SCALE,
        "b1": np.zeros(FF, np.float32),
        "W2": rng.standard_normal((FF, N), dtype=np.float32) * (1.0 / np.sqrt(FF)),
        "b2": np.zeros(N, np.float32),
    }
    out = kernel(**demo)
    print("out", out.shape, out.dtype, np.abs(out).mean())


# revision 12
# speedup vs baseline: 1.0224x; 1.0224x over previous
"""Distributed transformer-block kernel for one TRN2 chip (8 NeuronCores).

Reference computation (S=4096, N=1024):
    xn = LayerNorm(x) * g + b
    q,k,v = xn@Wq+bq, xn@Wk+bk, xn@Wv+bv
    w = softmax((k @ q.T) / sqrt(N), axis=-1)
    h = w @ v
    out = leaky_relu(h@W1+b1, 0.1) @ W2 + b2 + xn

Sharding: sequence-parallel. Each core owns 512 rows of x, computes its
q/k/v shard, all-gathers q and v (bf16), then computes its 512-row slice
of attention + FFN fully locally.

Layout strategy (picked so no big transposes are needed):
  - xn kept natural [512,1024] (residual) and transposed once to
    xnT [1024,512] bf16 (feeds every projection).
  - q, k produced transposed ([1024,512]); v produced natural.
  - logits computed TRANSPOSED: wT[j,i] = q_full @ k_loc.T, so the
    attention matmul hT[c,i] = sum_j v[j,c] * exp(wT)[j,i] uses v natural
    as lhsT and wT as rhs, and hT directly feeds FFN1 as rhs.
  - softmax denominator: exp accumulated with DVE, reduced over
    partitions with a ones-vector matmul. 1/sum applied at the FFN2
    epilogue via leaky_relu's positive homogeneity (bias terms are
    carried through scaled by sum via rank-1 matmul augmentation).

SBUF pressure is managed by entering/exiting tile pools at phase
boundaries (non-nested lifetimes, hence manual __enter__/__exit__).
"""

import sys

sys.path.insert(0, "/opt/trn_rl_repo")

import numpy as np
import ml_dtypes

import concourse.bass as bass
from concourse import bacc, tile, mybir
from concourse.bass import ts
from concourse.bass_utils import run_bass_kernel_spmd
from concourse.masks import make_identity

F32 = mybir.dt.float32
BF16 = mybir.dt.bfloat16
AF = mybir.ActivationFunctionType

P = 128
R = 8            # cores
S = 4096         # sequence
N = 1024         # hidden
FF = 4096        # ffn hidden
SL = S // R      # local rows (512)
NK = N // P      # 8 hidden chunks
NI = SL // P     # 4 local row chunks
NJ = S // P      # 32 global row chunks
NF = FF // P     # 32 ffn chunks
SCALE = 1.0 / np.sqrt(N).astype(np.float32)  # 0.03125
EPS = 1e-5

_cached = None


def _build():
    nc = bacc.Bacc("TRN2", target_bir_lowering=False, debug=False, num_devices=R)

    x_e = nc.declare_dram_parameter("x", [SL, N], F32, isOutput=False)
    g_e = nc.declare_dram_parameter("norm_g", [N], F32, isOutput=False)
    bn_e = nc.declare_dram_parameter("norm_b", [N], F32, isOutput=False)
    wq_e = nc.declare_dram_parameter("wq", [N, N], BF16, isOutput=False)
    bq_e = nc.declare_dram_parameter("bq", [N], F32, isOutput=False)
    wk_e = nc.declare_dram_parameter("wk", [N, N], BF16, isOutput=False)
    bk_e = nc.declare_dram_parameter("bk", [N], F32, isOutput=False)
    wv_e = nc.declare_dram_parameter("wv", [N, N], BF16, isOutput=False)
    bv_e = nc.declare_dram_parameter("bv", [N], BF16, isOutput=False)
    w1_e = nc.declare_dram_parameter("w1", [N, FF], BF16, isOutput=False)
    b1_e = nc.declare_dram_parameter("b1", [FF], BF16, isOutput=False)
    w2_e = nc.declare_dram_parameter("w2", [FF, N], BF16, isOutput=False)
    b2_e = nc.declare_dram_parameter("b2", [N], BF16, isOutput=False)
    out_e = nc.declare_dram_parameter("out", [SL, N], F32, isOutput=True)

    # collective bounce buffers
    agq_in = nc.dram_tensor("agq_in", [NK, P, SL], BF16)
    agq_out = nc.dram_tensor("agq_out", [R * NK, P, SL], BF16, addr_space="Shared")
    agv_in = nc.dram_tensor("agv_in", [NI, P, N], BF16)
    agv_out = nc.dram_tensor("agv_out", [R * NI, P, N], BF16, addr_space="Shared")

    rg = [list(range(R))]

    def enter(cm):
        return cm, cm.__enter__()

    def leave(cm):
        cm.__exit__(None, None, None)

    with tile.TileContext(nc) as tc:
        base_cm, base = enter(tc.tile_pool(name="base", bufs=1))

        # ---- whole-kernel constants / carriers ----
        ident = base.tile([P, P], BF16)
        make_identity(nc, ident)
        ones_row_b = base.tile([1, P], BF16)
        nc.gpsimd.memset(ones_row_b[:], 1.0)
        ones_col_f = base.tile([P, 1], F32)
        nc.gpsimd.memset(ones_col_f[:], 1.0)
        one_f = base.tile([1, 1], F32)
        nc.gpsimd.memset(one_f[:], 1.0)
        zero_col = base.tile([P, 1], F32)
        nc.gpsimd.memset(zero_col[:], 0.0)
        eps_col = base.tile([P, 1], F32)
        nc.gpsimd.memset(eps_col[:], EPS)

        xn_sb = base.tile([P, NI * N], BF16)    # normed x, natural layout (residual)
        sum_row_f = base.tile([1, SL], F32)
        sum_row_b = base.tile([1, SL], BF16)
        recip_col = base.tile([P, NI], F32)

        # =========== Phase 0: layernorm + transpose ===========
        xnT_cm, xnTp = enter(tc.tile_pool(name="xnTp", bufs=1, side="left"))
        xnT_sb = xnTp.tile([P, NK * SL], BF16)

        # per-partition views of the LN affine for the transposed layout
        g_col = base.tile([P, NK], F32)
        nc.sync.dma_start(g_col[:], g_e[:].rearrange("(m p) -> p m", p=P))
        b_col = base.tile([P, NK], F32)
        nc.sync.dma_start(b_col[:], bn_e[:].rearrange("(m p) -> p m", p=P))

        with (
            tc.tile_pool(name="xs", bufs=4) as xs,
            tc.tile_pool(name="ln", bufs=4) as ln,
            tc.tile_pool(name="tpsum", bufs=4, space="PSUM") as tpsum,
        ):
            for i in range(NI):
                xt = xs.tile([P, N], F32, tag="xt")
                nc.sync.dma_start(xt[:], x_e[ts(i, P), :])
                sum_t = ln.tile([P, 1], F32, tag="sum")
                nc.vector.reduce_sum(sum_t[:], xt[:], axis=mybir.AxisListType.X)
                sq_scr = xs.tile([P, N], BF16, tag="sq")
                sumsq_t = ln.tile([P, 1], F32, tag="sumsq")
                nc.scalar.activation(sq_scr[:], xt[:], AF.Square, bias=zero_col[:], accum_out=sumsq_t[:])
                mu_t = ln.tile([P, 1], F32, tag="mu")
                nc.vector.tensor_scalar_mul(mu_t[:], sum_t[:], 1.0 / N)
                var_t = ln.tile([P, 1], F32, tag="var")
                nc.vector.tensor_scalar_mul(var_t[:], sumsq_t[:], 1.0 / N)
                musq_t = ln.tile([P, 1], F32, tag="musq")
                nc.vector.tensor_mul(musq_t[:], mu_t[:], mu_t[:])
                nc.vector.tensor_sub(var_t[:], var_t[:], musq_t[:])
                std_t = ln.tile([P, 1], F32, tag="std")
                nc.scalar.activation(std_t[:], var_t[:], AF.Sqrt, bias=eps_col[:])
                rstd_t = ln.tile([P, 1], F32, tag="rstd")
                nc.vector.reciprocal(rstd_t[:], std_t[:])
                nmr_t = ln.tile([P, 1], F32, tag="nmr")
                nc.vector.tensor_mul(nmr_t[:], mu_t[:], rstd_t[:])
                nc.vector.tensor_scalar_mul(nmr_t[:], nmr_t[:], -1.0)
                # xn_sb holds z = (x-mu)*rstd (bf16); affine for the residual
                # is applied in-place later, off the critical path
                xn_i = xn_sb[:, ts(i, N)]
                nc.scalar.activation(xn_i, xt[:], AF.Identity, scale=rstd_t[:], bias=nmr_t[:])
                for k in range(NK):
                    pt = tpsum.tile([P, P], BF16, tag="pt")
                    nc.tensor.transpose(pt[:], xn_sb[:, i * N + k * P : i * N + (k + 1) * P], ident[:])
                    # affine fused here: in transposed layout g,b are per-partition
                    nc.scalar.activation(
                        xnT_sb[:, k * SL + i * P : k * SL + (i + 1) * P], pt[:], AF.Identity,
                        scale=g_col[:, k : k + 1], bias=b_col[:, k : k + 1],
                    )
        

        # =========== Phase 1: q/k/v projections + all-gathers ===========
        kT_cm, kTp = enter(tc.tile_pool(name="kTp", bufs=1, side="right"))
        kT_sb = kTp.tile([P, NK * SL], BF16)

        qkv_cm, qkv = enter(tc.tile_pool(name="qkv", bufs=1, side="right"))
        bq_col = qkv.tile([P, NK], F32)
        nc.sync.dma_start(bq_col[:], bq_e[:].rearrange("(m p) -> p m", p=P))
        bk_col = qkv.tile([P, NK], F32)
        nc.sync.dma_start(bk_col[:], bk_e[:].rearrange("(m p) -> p m", p=P))
        bv_row = qkv.tile([1, N], BF16)
        nc.sync.dma_start(bv_row[:1, :], bv_e[:].rearrange("(a n) -> a n", a=1))
        wq_sb = [qkv.tile([P, N], BF16, tag=f"wq{k}", name=f"wq{k}") for k in range(NK)]
        wk_sb = [qkv.tile([P, N], BF16, tag=f"wk{k}", name=f"wk{k}") for k in range(NK)]
        wv_sb = [qkv.tile([P, N], BF16, tag=f"wv{k}", name=f"wv{k}") for k in range(NK)]
        qT_sb = qkv.tile([P, NK * SL], BF16)
        v_sb = qkv.tile([P, NI * N], BF16)
        for k in range(NK):
            nc.sync.dma_start(wq_sb[k][:], wq_e[ts(k, P), :])
        for k in range(NK):
            nc.sync.dma_start(wk_sb[k][:], wk_e[ts(k, P), :])
        for k in range(NK):
            nc.sync.dma_start(wv_sb[k][:], wv_e[ts(k, P), :])

        with tc.tile_pool(name="qpsum", bufs=3, space="PSUM") as qpsum:
            # q (transposed layout), then its all-gather right away
            for m in range(NK):
                pq = qpsum.tile([P, SL], F32, tag="pq")
                for k in range(NK):
                    nc.tensor.matmul(
                        pq[:],
                        wq_sb[k][:, ts(m, P)],
                        xnT_sb[:, ts(k, SL)],
                        start=(k == 0),
                        stop=(k == NK - 1),
                    )
                nc.scalar.activation(
                    qT_sb[:, ts(m, SL)], pq[:], AF.Identity, bias=bq_col[:, m : m + 1]
                )
            for m in range(NK):
                nc.gpsimd.dma_start(agq_in[m], qT_sb[:, ts(m, SL)])
            nc.gpsimd.collective_compute(
                "AllGather", mybir.AluOpType.bypass, replica_groups=rg,
                ins=[agq_in[:]], outs=[agq_out[:]],
            )

            # k (transposed layout, stays local)
            for m in range(NK):
                pk = qpsum.tile([P, SL], F32, tag="pq")
                for k in range(NK):
                    nc.tensor.matmul(
                        pk[:],
                        wk_sb[k][:, ts(m, P)],
                        xnT_sb[:, ts(k, SL)],
                        start=(k == 0),
                        stop=(k == NK - 1),
                    )
                nc.scalar.activation(
                    kT_sb[:, ts(m, SL)], pk[:], AF.Identity, bias=bk_col[:, m : m + 1]
                )

            # v (natural layout) + its all-gather
            for i in range(NI):
                for cb in range(2):
                    pv = qpsum.tile([P, 512], F32, tag="pq")
                    nc.tensor.matmul(
                        pv[:], ones_row_b[:], bv_row[:1, ts(cb, 512)],
                        start=True, stop=False,
                    )
                    for k in range(NK):
                        nc.tensor.matmul(
                            pv[:],
                            xnT_sb[:, k * SL + i * P : k * SL + (i + 1) * P],
                            wv_sb[k][:, ts(cb, 512)],
                            start=False,
                            stop=(k == NK - 1),
                        )
                    nc.scalar.activation(
                        v_sb[:, i * N + cb * 512 : i * N + (cb + 1) * 512], pv[:], AF.Copy
                    )
            for i in range(NI):
                nc.gpsimd.dma_start(agv_in[i], v_sb[:, ts(i, N)])
            nc.gpsimd.collective_compute(
                "AllGather", mybir.AluOpType.bypass, replica_groups=rg,
                ins=[agv_in[:]], outs=[agv_out[:]],
            )
        leave(qkv_cm)
        leave(xnT_cm)

        # W1 resident; emitted here so it prefetches during attention
        w1_cm, w1p = enter(tc.tile_pool(name="w1p", bufs=1, side="left"))
        w1_sb = [w1p.tile([P, FF], BF16, tag=f"w1{c}", name=f"w1{c}") for c in range(NK)]
        for c in range(NK):
            nc.gpsimd.dma_start(w1_sb[c][:], w1_e[ts(c, P), :])
        b1_row = w1p.tile([1, FF], BF16)
        nc.gpsimd.dma_start(b1_row[:1, :], b1_e[:].rearrange("(a n) -> a n", a=1))

        # =========== Phase 2: logits (transposed) + exp + running sum ===========
        wT_cm, wTp = enter(tc.tile_pool(name="wTp", bufs=1, side="left"))
        wT_sb = wTp.tile([P, NJ * SL], BF16)
        acc = wTp.tile([P, SL], F32)
        nc.vector.memset(acc[:], 0.0)
        with (
            tc.tile_pool(name="qf", bufs=3) as qfp,
            tc.tile_pool(name="wpsum", bufs=3, space="PSUM") as wpsum,
        ):
            for rank in range(R):
                qf = qfp.tile([P, NK * SL], BF16, tag="qf")
                for n in range(NK):
                    nc.sync.dma_start(qf[:, ts(n, SL)], agq_out[rank * NK + n])
                for sub in range(NI):
                    jc = rank * NI + sub
                    pw = wpsum.tile([P, SL], F32, tag="pw")
                    for n in range(NK):
                        nc.tensor.matmul(
                            pw[:],
                            qf[:, n * SL + sub * P : n * SL + (sub + 1) * P],
                            kT_sb[:, ts(n, SL)],
                            start=(n == 0),
                            stop=(n == NK - 1),
                        )
                    nc.scalar.activation(
                        wT_sb[:, ts(jc, SL)], pw[:], AF.Exp, scale=float(SCALE), bias=zero_col[:]
                    )
                    nc.vector.tensor_add(acc[:], acc[:], wT_sb[:, ts(jc, SL)])
        leave(kT_cm)

        # deferred residual affine: xn_sb = z*g + b, done during idle DVE time
        with (
            tc.tile_pool(name="bc", bufs=1, side="left") as bc,
            tc.tile_pool(name="bpsum", bufs=2, space="PSUM") as bpsum,
        ):
            ones_row_f = bc.tile([1, P], F32)
            nc.gpsimd.memset(ones_row_f[:], 1.0)
            g_row = bc.tile([1, N], F32)
            nc.sync.dma_start(g_row[:1, :], g_e[:].rearrange("(a n) -> a n", a=1))
            b_row = bc.tile([1, N], F32)
            nc.sync.dma_start(b_row[:1, :], bn_e[:].rearrange("(a n) -> a n", a=1))
            g_bcast = bc.tile([P, N], F32)
            b_bcast = bc.tile([P, N], F32)
            for vec_row, bcast in ((g_row, g_bcast), (b_row, b_bcast)):
                for blk in range(2):
                    pb = bpsum.tile([P, 512], F32, tag="pb")
                    nc.tensor.matmul(pb[:], ones_row_f[:], vec_row[:1, ts(blk, 512)])
                    nc.vector.tensor_copy(bcast[:, ts(blk, 512)], pb[:])
            for i in range(NI):
                xn_i = xn_sb[:, ts(i, N)]
                nc.vector.tensor_mul(xn_i, xn_i, g_bcast[:])
                nc.vector.tensor_add(xn_i, xn_i, b_bcast[:])

        # =========== Phase 3: hT accumulation over all j ===========
        mid_cm, midp = enter(tc.tile_pool(name="midp", bufs=1, side="right"))
        hT_sb = midp.tile([P, NK * SL], BF16)
        ff1T_sb = midp.tile([P, NF * SL], BF16)
        with (
            tc.tile_pool(name="vstream", bufs=6) as vsp,
            tc.tile_pool(name="hpsum", bufs=1, space="PSUM") as hpsum,
        ):
            ph = [hpsum.tile([P, SL], F32, tag=f"ph{c}", name=f"ph{c}") for c in range(NK)]
            for j in range(NJ):
                vt = vsp.tile([P, N], BF16, tag="vt")
                nc.sync.dma_start(vt[:], agv_out[j])
                for c in range(NK):
                    nc.tensor.matmul(
                        ph[c][:],
                        vt[:, ts(c, P)],
                        wT_sb[:, ts(j, SL)],
                        start=(j == 0),
                        stop=(j == NJ - 1),
                    )
            for c in range(NK):
                if c % 2 == 0:
                    nc.scalar.activation(hT_sb[:, ts(c, SL)], ph[c][:], AF.Copy)
                else:
                    nc.vector.tensor_copy(hT_sb[:, ts(c, SL)], ph[c][:])
        # sumexp finalize: PE cost is tiny and overlaps the hT evacuations
        with tc.tile_pool(name="spsum", bufs=2, space="PSUM") as spsum:
            ps = spsum.tile([1, SL], F32, tag="ps")
            nc.tensor.matmul(ps[:], ones_col_f[:], acc[:])
            nc.vector.tensor_copy(sum_row_f[:1, :], ps[:1, :])
            nc.scalar.activation(sum_row_b[:1, :], ps[:1, :], AF.Copy)
            for ic in range(NI):
                pr = spsum.tile([P, 1], F32, tag="pr")
                nc.tensor.matmul(pr[:], sum_row_f[:1, ts(ic, P)], one_f[:1, :])
                nc.vector.reciprocal(recip_col[:, ic : ic + 1], pr[:])
        leave(wT_cm)

        # =========== Phase 4: FFN1 (transposed out, leaky via homogeneity) ===========
        with tc.tile_pool(name="fpsum", bufs=3, space="PSUM") as fpsum:
            for f in range(NF):
                pf = fpsum.tile([P, SL], F32, tag="pf")
                nc.tensor.matmul(
                    pf[:], b1_row[:1, ts(f, P)], sum_row_b[:1, :],
                    start=True, stop=False,
                )
                for c in range(NK):
                    nc.tensor.matmul(
                        pf[:],
                        w1_sb[c][:, ts(f, P)],
                        hT_sb[:, ts(c, SL)],
                        start=False,
                        stop=(c == NK - 1),
                    )
                nc.scalar.activation(ff1T_sb[:, ts(f, SL)], pf[:], AF.Lrelu, alpha=0.1, bias=zero_col[:])
        leave(w1_cm)

        # =========== Phase 5: FFN2 + epilogue (scale, bias, residual) ===========
        with (
            tc.tile_pool(name="ph5", bufs=1) as ph5,
            tc.tile_pool(name="w2s", bufs=8) as w2s,
            tc.tile_pool(name="outp", bufs=3) as outp,
            tc.tile_pool(name="opsum", bufs=1, space="PSUM") as opsum,
        ):
            b2_row = ph5.tile([1, N], BF16)
            nc.sync.dma_start(b2_row[:1, :], b2_e[:].rearrange("(a n) -> a n", a=1))
            po = [
                opsum.tile([P, 512], F32, tag=f"po{i}", name=f"po{i}")
                for i in range(NI * 2)
            ]
            for ic in range(NI):
                for mb in range(2):
                    nc.tensor.matmul(
                        po[ic * 2 + mb][:],
                        sum_row_b[:1, ts(ic, P)],
                        b2_row[:1, ts(mb, 512)],
                        start=True, stop=False,
                    )
            for f in range(NF):
                w2t = w2s.tile([P, N], BF16, tag="w2t")
                nc.sync.dma_start(w2t[:], w2_e[ts(f, P), :])
                for ic in range(NI):
                    for mb in range(2):
                        nc.tensor.matmul(
                            po[ic * 2 + mb][:],
                            ff1T_sb[:, f * SL + ic * P : f * SL + (ic + 1) * P],
                            w2t[:, ts(mb, 512)],
                            start=False,
                            stop=(f == NF - 1),
                        )
            for ic in range(NI):
                for mb in range(2):
                    ot = outp.tile([P, 512], F32, tag="ot")
                    nc.vector.scalar_tensor_tensor(
                        ot[:],
                        po[ic * 2 + mb][:],
                        recip_col[:, ic : ic + 1],
                        xn_sb[:, ic * N + mb * 512 : ic * N + (mb + 1) * 512],
                        op0=mybir.AluOpType.mult,
                        op1=mybir.AluOpType.add,
                    )
                    nc.sync.dma_start(out_e[ts(ic, P), ts(mb, 512)], ot[:])
        leave(mid_cm)
        leave(base_cm)

    nc.compile()
    return nc


def _get_nc():
    global _cached
    if _cached is None:
        _cached = _build()
    return _cached


def kernel(**inputs):
    nc = _get_nc()
    bf = lambda a: np.asarray(a, dtype=np.float32).astype(ml_dtypes.bfloat16)
    f = lambda a: np.ascontiguousarray(np.asarray(a, dtype=np.float32))
    x = f(inputs["x"])
    common = {
        "norm_g": f(inputs["norm_g"]),
        "norm_b": f(inputs["norm_b"]),
        "wq": bf(inputs["Wq"]),
        "bq": f(inputs["bq"]),
        "wk": bf(inputs["Wk"]),
        "bk": f(inputs["bk"]),
        "wv": bf(inputs["Wv"]),
        "bv": bf(inputs["bv"]),
        "w1": bf(inputs["W1"]),
        "b1": bf(inputs["b1"]),
        "w2": bf(inputs["W2"]),
        "b2": bf(inputs["b2"]),
    }
    in_maps = [dict(common, x=np.ascontiguousarray(x[r * SL : (r + 1) * SL])) for r in range(R)]
    res = run_bass_kernel_spmd(nc, in_maps, list(range(R)))
    return np.concatenate([res.results[r]["out"] for r in range(R)], axis=0)


if __name__ == "__main__":
    rng = np.random.default_rng(0)
    demo = {
        "x": rng.standard_normal((S, N), dtype=np.float32),
        "norm_g": np.ones(N, np.float32),
        "norm_b": np.zeros(N, np.float32),
        "Wq": rng.standard_normal((N, N), dtype=np.float32) * SCALE,
        "bq": np.zeros(N, np.float32),
        "Wk": rng.standard_normal((N, N), dtype=np.float32) * SCALE,
        "bk": np.zeros(N, np.float32),
        "Wv": rng.standard_normal((N, N), dtype=np.float32) * SCALE,
        "bv": np.zeros(N, np.float32),
        "W1": rng.standard_normal((N, FF), dtype=np.float32) * SCALE,
        "b1": np.zeros(FF, np.float32),
        "W2": rng.standard_normal((FF, N), dtype=np.float32) * (1.0 / np.sqrt(FF)),
        "b2": np.zeros(N, np.float32),
    }
    out = kernel(**demo)
    print("out", out.shape, out.dtype, np.abs(out).mean())
